# revision 3
# baseline (speedup 1.0000x reference)
"""Multi-head attention (B=2, S=4096, D=768, H=12) on 8 TRN2 NeuronCores.

Sharding: 24 (batch, head) pairs -> 3 heads per core. Cores 0-3 take batch 0,
cores 4-7 take batch 1. Each core computes q/k/v projections for its 3 heads,
flash-style attention (scores kept transposed [kv, q]), and a partial output
projection over its 192 contraction rows. The host sums the 4 partial outputs
per batch and adds the output bias.

Softmax exp is split across two engines: the Scalar engine (ACT LUT exp) and
a custom Vector-engine op EXP8C that evaluates exp(8v) as a factored minimax
cubic for e^v followed by three squarings. Scores are produced pre-scaled
(host folds g/64 into wq/bq, g = c3^(1/3) of the cubic) so both paths read the
same PSUM tile: ACT applies scale 8/g in its free affine; the DVE op's cubic
constants absorb g.

The k/v projections are pipelined into the first q-chunk's attention pass
(per-seq-block just-in-time), so the Scalar/Vector engines start exp work
almost immediately instead of idling through a serial projection prefix.
Softmax denominators ride a ones-column through the PV matmul, are broadcast
across partitions with a tiny K=1 matmul (no DRAM bounce), inverted with the
fast DVE reciprocal, and applied to the context before the output projection.
"""

import sys

sys.path.insert(0, "/opt/trn_rl_repo")

import numpy as np  # noqa: E402

from concourse import bacc, bass, mybir, tile  # noqa: E402
from concourse import dve_ops  # noqa: E402
from concourse.bass_utils import run_bass_kernel_spmd  # noqa: E402
from concourse.dve_spec import C0, C1, C2, Spec, Src0, lower  # noqa: E402
from concourse.dve_uop import DveOpSpec  # noqa: E402

S = 4096
DM = 768
DK = 64
HPC = 3  # heads per core
NC_CORES = 8
KC = DM // 128  # 6 contraction chunks for projections
NSB = S // 512  # 8 seq blocks (projection N / attention q chunks)
NKV = S // 128  # 32 kv chunks
NG = 16  # kv groups of 2 per (head, q-chunk)

# exp(8v) ~ ((vt-EA)(vt^2+EB*vt+EC))^8 with vt = GEXP*v, v = s_raw/64.
# Minimax cubic c(v) ~ e^v on [-0.5, 0.5] (rel err 3.2e-4), factored as
# c3*(v-rho)(v^2+b*v+cq); GEXP = c3^(1/3) distributed into the factors.
GEXP = 0.5480315395002889
EA = -0.9028239  # GEXP * rho
EB = 0.79605097  # GEXP * b
EC = 1.1073068  # GEXP^2 * cq
ACT_SCALE = 8.0 / GEXP  # Scalar-engine exp free-affine scale
QPRE = GEXP / 64.0  # host-side wq/bq prescale

F16 = mybir.dt.float16
F32 = mybir.dt.float32


def _exp8c_ref(in0, in1, s0, s1, imm2):
    x = np.asarray(in0, np.float32)
    fa = (x - np.float32(s0)).astype(np.float32)
    f5 = (x * (x + np.float32(s1)) + np.float32(imm2)).astype(np.float32)
    p = (fa * f5).astype(np.float32)
    for _ in range(3):
        p = (p * p).astype(np.float32)
    return p


def _register_exp8c():
    name = "EXP8C_ANT"
    if name in dve_ops._SUB_OPCODE_FOR_NAME:
        return next(op for op in dve_ops.OPS if op.name == name)
    fa = Src0 - C0
    f5 = (Src0 * (Src0 + C1)) + C2
    f6 = fa * f5
    s1 = f6 * f6
    s2 = s1 * s1
    body = s2 * s2
    spec = Spec(body=body, reference=_exp8c_ref)
    row = max(dve_ops._SUB_OPCODE_FOR_NAME.values()) + 1
    assert row < 0x20, "no free custom-DVE opcode row"
    shas = {}
    for ver in ("v3", "v4"):
        uops = lower(spec, ver=ver)
        shas[ver] = DveOpSpec(
            name=name, opcode=row, uops=uops, rd1_en=False
        ).sha(ver)
    op = dve_ops.DveOp(name, spec, subdim=False, uops_sha=shas)
    dve_ops.OPS.append(op)
    dve_ops.CUSTOM_DVE_SPECS[name] = spec
    dve_ops._SUB_OPCODE_FOR_NAME[name] = row
    return op


EXP8C = _register_exp8c()

# groups handled by the Vector engine (5 of 16): g % 3 == 2
DVE_GROUPS = frozenset(g for g in range(NG) if g % 3 == 2)


def _emit(tc):
    nc = tc.nc
    qTx = nc.dram_tensor("qTx", [KC, NSB, 128, 512], F16, kind="ExternalInput").ap()
    kTx = nc.dram_tensor("kTx", [KC, NSB, 128, 512], F16, kind="ExternalInput").ap()
    vTx = nc.dram_tensor("vTx", [KC, NSB, 128, 512], F16, kind="ExternalInput").ap()
    wqT = nc.dram_tensor("wqT", [DM, HPC * DK], F16, kind="ExternalInput").ap()
    wkT = nc.dram_tensor("wkT", [DM, HPC * DK], F16, kind="ExternalInput").ap()
    wvT = nc.dram_tensor("wvT", [DM, HPC * DK], F16, kind="ExternalInput").ap()
    wo01 = nc.dram_tensor("wo01", [128, DM], F16, kind="ExternalInput").ap()
    wo2d_d = nc.dram_tensor("wo2d", [128, DM], F16, kind="ExternalInput").ap()
    bq = nc.dram_tensor("bq", [HPC * DK, 1], F32, kind="ExternalInput").ap()
    bk = nc.dram_tensor("bk", [HPC * DK, 1], F32, kind="ExternalInput").ap()
    bv = nc.dram_tensor("bv", [HPC * DK, 1], F32, kind="ExternalInput").ap()
    out_p = nc.dram_tensor("out_p", [S, DM], F32, kind="ExternalOutput").ap()

    with (
        tc.tile_pool(name="const", bufs=1) as const,
        tc.tile_pool(name="heads", bufs=1) as heads,
        tc.tile_pool(name="xts", bufs=12) as xts,
        tc.tile_pool(name="work", bufs=3) as work,
        tc.tile_pool(name="norm", bufs=2) as norm,
        tc.tile_pool(name="sp", bufs=2, space=bass.MemorySpace.PSUM) as sp,
        tc.tile_pool(name="cxp", bufs=2, space=bass.MemorySpace.PSUM) as cxp,
        tc.tile_pool(name="aux", bufs=2, space=bass.MemorySpace.PSUM) as aux,
    ):
        # ---- constants -------------------------------------------------
        w_q = const.tile([128, KC, HPC * DK], F16, tag="w_q")
        w_k = const.tile([128, KC, HPC * DK], F16, tag="w_k")
        w_v = const.tile([128, KC, HPC * DK], F16, tag="w_v")
        wo01_t = const.tile([128, DM], F16, tag="wo01")
        wo2d_t = const.tile([128, DM], F16, tag="wo2d")
        bq01 = const.tile([128, 1], F32, tag="bq01")
        bq2 = const.tile([DK, 1], F32, tag="bq2")
        bk01 = const.tile([128, 1], F32, tag="bk01")
        bk2 = const.tile([DK, 1], F32, tag="bk2")
        bvb = const.tile([128, HPC * DK], F32, tag="bvb")
        ones16 = const.tile([1, DK], F16, tag="ones16")
        bv_bcast = bass.AP(
            tensor=bv.tensor, offset=bv.offset, ap=[[0, 128]] + list(bv.ap)
        )

        nc.sync.dma_start(w_k[:], wkT.rearrange("(c p) m -> p c m", p=128))
        nc.sync.dma_start(bk01[:], bk[0:128, :])
        nc.sync.dma_start(bk2[:], bk[128:192, :])
        nc.sync.dma_start(w_q[:], wqT.rearrange("(c p) m -> p c m", p=128))
        nc.sync.dma_start(bq01[:], bq[0:128, :])
        nc.sync.dma_start(bq2[:], bq[128:192, :])
        nc.vector.memset(ones16[:], 1.0)

        # preload the exp activation table early
        warm = const.tile([1, 1], F32, tag="warm")
        nc.vector.memset(warm[:], 0.0)
        nc.scalar.activation(warm[:], warm[:], mybir.ActivationFunctionType.Exp)

        # ---- per-head persistent tensors ------------------------------
        # qT2/kT2: [128, S] fp16, rows 0:64 and 64:128 duplicated so
        # row-tiled matmul pairs can stream rhs from both partition halves.
        qT2 = [heads.tile([128, S], F16, tag=f"qT2_{h}", name=f"qT2_{h}") for h in range(HPC)]
        kT2 = [heads.tile([128, S], F16, tag=f"kT2_{h}", name=f"kT2_{h}") for h in range(HPC)]
        # v_aug: [128, NKV*65]; group c cols [65c, 65c+64) = v rows of
        # kv-chunk c, col 65c+64 = 1.0 (denominator column).
        v_aug = [heads.tile([128, NKV * 65], F16, tag=f"v_aug_{h}", name=f"v_aug_{h}") for h in range(HPC)]
        # normalized context, transposed: ctx01 rows 0:64 = head 0, rows
        # 64:128 = head 1; ctx2d rows 0:64 == rows 64:128 = head 2
        # (duplicated for the paired output projection).
        ctx01 = heads.tile([128, S], F16, tag="ctx01")
        ctx2d = heads.tile([128, S], F16, tag="ctx2d")

        for h in range(HPC):
            nc.vector.memset(v_aug[h][:], 1.0)

        nc.sync.dma_start(w_v[:], wvT.rearrange("(c p) m -> p c m", p=128))
        nc.sync.dma_start(bvb[:], bv_bcast)
        nc.sync.dma_start(wo01_t[:], wo01)
        nc.sync.dma_start(wo2d_t[:], wo2d_d)

        # ---- input tile DMA helpers -----------------------------------
        kx_tiles = {}
        vx_tiles = {}
        qx_tiles = {}

        def kx_dma(s):
            kx_tiles[s] = []
            for kc in range(KC):
                t = xts.tile([128, 512], F16, tag="kx", bufs=12, name=f"kx_{s}_{kc}")
                nc.sync.dma_start(t[:], kTx[kc, s])
                kx_tiles[s].append(t)

        def vx_dma(s):
            vx_tiles[s] = []
            for kc in range(KC):
                t = xts.tile([128, 512], F16, tag="vx", bufs=12, name=f"vx_{s}_{kc}")
                nc.sync.dma_start(t[:], vTx[kc, s])
                vx_tiles[s].append(t)

        def qx_dma(qc):
            qx_tiles[qc] = []
            for kc in range(KC):
                t = xts.tile([128, 512], F16, tag="qx", bufs=12, name=f"qx_{qc}_{kc}")
                nc.sync.dma_start(t[:], qTx[kc, qc])
                qx_tiles[qc].append(t)

        # ---- projection helpers ---------------------------------------
        def kproj(s):
            # heads 0/1: one aux pass, M=128
            sq = bass.ts(s, 512)
            t01 = aux.tile([128, 512], F32, tag="aux", name=f"k01_{s}")
            for kc in range(KC):
                nc.tensor.matmul(
                    t01[:], w_k[:, kc, 0:128], kx_tiles[s][kc][:],
                    start=(kc == 0), stop=(kc == KC - 1),
                )
            nc.vector.tensor_scalar_add(kT2[0][0:64, sq], t01[0:64, :], bk01[0:64, :])
            nc.vector.tensor_scalar_add(kT2[0][64:128, sq], t01[0:64, :], bk01[0:64, :])
            nc.vector.tensor_scalar_add(kT2[1][0:64, sq], t01[64:128, :], bk01[64:128, :])
            nc.vector.tensor_scalar_add(kT2[1][64:128, sq], t01[64:128, :], bk01[64:128, :])
            # head 2: second aux pass, M=64
            t2 = aux.tile([128, 512], F32, tag="aux", name=f"k2_{s}")
            for kc in range(KC):
                nc.tensor.matmul(
                    t2[0:64, :], w_k[:, kc, 128:192], kx_tiles[s][kc][:],
                    start=(kc == 0), stop=(kc == KC - 1),
                )
            nc.vector.tensor_scalar_add(kT2[2][0:64, sq], t2[0:64, :], bk2[:])
            nc.vector.tensor_scalar_add(kT2[2][64:128, sq], t2[0:64, :], bk2[:])

        def vproj(c):
            # kv chunk c: [128 seq, 192 dims] via aux; seq slice within sb
            s, ss = c // 4, c % 4
            vp = aux.tile([128, 512], F32, tag="aux", name=f"vp_{c}")
            for kc in range(KC):
                nc.tensor.matmul(
                    vp[:, 0:HPC * DK],
                    vx_tiles[s][kc][:, bass.ts(ss, 128)],
                    w_v[:, kc, :],
                    start=(kc == 0), stop=(kc == KC - 1),
                )
            g = c * 65
            for h in range(HPC):
                nc.vector.tensor_add(
                    v_aug[h][:, g : g + 64],
                    vp[:, bass.ts(h, 64)],
                    bvb[:, bass.ts(h, 64)],
                )

        def qproj(qc):
            sq = bass.ts(qc, 512)
            t01 = aux.tile([128, 512], F32, tag="aux", name=f"q01_{qc}")
            for kc in range(KC):
                nc.tensor.matmul(
                    t01[:], w_q[:, kc, 0:128], qx_tiles[qc][kc][:],
                    start=(kc == 0), stop=(kc == KC - 1),
                )
            nc.vector.tensor_scalar_add(qT2[0][0:64, sq], t01[0:64, :], bq01[0:64, :])
            nc.vector.tensor_scalar_add(qT2[0][64:128, sq], t01[0:64, :], bq01[0:64, :])
            nc.vector.tensor_scalar_add(qT2[1][0:64, sq], t01[64:128, :], bq01[64:128, :])
            nc.vector.tensor_scalar_add(qT2[1][64:128, sq], t01[64:128, :], bq01[64:128, :])
            t2 = aux.tile([128, 512], F32, tag="aux", name=f"q2_{qc}")
            for kc in range(KC):
                nc.tensor.matmul(
                    t2[0:64, :], w_q[:, kc, 128:192], qx_tiles[qc][kc][:],
                    start=(kc == 0), stop=(kc == KC - 1),
                )
            nc.vector.tensor_scalar_add(qT2[2][0:64, sq], t2[0:64, :], bq2[:])
            nc.vector.tensor_scalar_add(qT2[2][64:128, sq], t2[0:64, :], bq2[:])

        # ---- attention helpers ----------------------------------------
        ctx_state = {}

        def att_group(h, qc, g):
            sq = bass.ts(qc, 512)
            kv = 2 * g
            if g == 0:
                ctx_state[(h, qc)] = cxp.tile([128, 512], F32, tag="cx", name=f"cx_{h}_{qc}")
            ctx = ctx_state[(h, qc)][0:65, :]
            sT = sp.tile([128, 1024], F32, tag="sT", name=f"sT_{h}_{qc}_{g}")
            for j in range(2):
                lo = 64 * j
                nc.tensor.matmul(
                    sT[:, bass.ts(j, 512)],
                    kT2[h][lo : lo + 64, bass.ts(kv + j, 128)],
                    qT2[h][lo : lo + 64, sq],
                )
            pt = work.tile([128, 1024], F16, tag="pt", bufs=6)
            if g in DVE_GROUPS:
                nc.vector._custom_dve(
                    EXP8C, out=pt[:], in0=sT[:], s0=EA, s1=EB, imm2=EC
                )
            else:
                nc.scalar.activation(
                    pt[:], sT[:], mybir.ActivationFunctionType.Exp, scale=ACT_SCALE
                )
            for j in range(2):
                gg = (kv + j) * 65
                nc.tensor.matmul(
                    ctx,
                    v_aug[h][:, gg : gg + 65],
                    pt[:, bass.ts(j, 512)],
                    start=(g == 0 and j == 0),
                    stop=(g == NG - 1 and j == 1),
                )

        def normalize(h, qc):
            # denominator row -> [1,512] f16 -> K=1 matmul broadcast to 64
            # partitions -> fast reciprocal -> scale ctx into SBUF fp16.
            sq = bass.ts(qc, 512)
            ctx = ctx_state.pop((h, qc))
            den16 = norm.tile([1, 512], F16, tag="den16")
            nc.vector.tensor_copy(den16[:], ctx[64:65, :])
            dbc = aux.tile([128, 512], F32, tag="aux", name=f"dbc_{h}_{qc}")
            nc.tensor.matmul(dbc[0:64, :], ones16[:], den16[:])
            rec = norm.tile([64, 512], F32, tag="rec")
            nc.vector.reciprocal_approx_fast(out=rec[:], in_=dbc[0:64, :])
            if h == 0:
                nc.vector.tensor_mul(ctx01[0:64, sq], ctx[0:64, :], rec[:])
            elif h == 1:
                nc.vector.tensor_mul(ctx01[64:128, sq], ctx[0:64, :], rec[:])
            else:
                nc.vector.tensor_mul(ctx2d[0:64, sq], ctx[0:64, :], rec[:])
                nc.vector.tensor_mul(ctx2d[64:128, sq], ctx[0:64, :], rec[:])

        def op_pair(qc, i):
            # chains 2i and 2i+1 of q-chunk qc's output projection; the two
            # head-2 matmuls (K=64) run as a concurrent row-tiled pair.
            chains = []
            for j in range(2):
                ci = 2 * i + j
                qs, half = ci // 2, ci % 2
                n0, nw = (0, 512) if half == 0 else (512, 256)
                qsl = bass.ds(qc * 512 + qs * 128, 128)
                op = aux.tile([128, 512], F32, tag="aux", name=f"op_{qc}_{ci}")
                nc.tensor.matmul(
                    op[:, 0:nw], ctx01[:, qsl], wo01_t[:, n0 : n0 + nw],
                    start=True, stop=False,
                )
                chains.append((op, qsl, n0, nw))
            for j in range(2):
                op, qsl, n0, nw = chains[j]
                lo = 64 * j
                nc.tensor.matmul(
                    op[:, 0:nw],
                    ctx2d[lo : lo + 64, qsl],
                    wo2d_t[lo : lo + 64, n0 : n0 + nw],
                    start=False, stop=True,
                )
            for j in range(2):
                op, qsl, n0, nw = chains[j]
                ob = work.tile([128, 512], F32, tag="ob", bufs=3)
                nc.vector.tensor_copy(ob[:, 0:nw], op[:, 0:nw])
                nc.sync.dma_start(out_p[qsl, n0 : n0 + nw], ob[:, 0:nw])

        def qdup(qc):
            sq = bass.ts(qc, 512)
            for h in range(HPC):
                nc.sync.dma_start(qT2[h][64:128, sq], qT2[h][0:64, sq])

        # ---- qc 0: JIT k/v projection + attention h0 -------------------
        qx_dma(0)
        kx_dma(0)
        vx_dma(0)
        qproj(0)
        qdup(0)
        for s in range(NSB):
            if s + 1 < NSB:
                kx_dma(s + 1)
                vx_dma(s + 1)
            kproj(s)
            for c in range(4 * s, 4 * s + 4):
                vproj(c)
            att_group(0, 0, 2 * s)
            att_group(0, 0, 2 * s + 1)
        normalize(0, 0)
        qx_dma(1)
        for g in range(NG):
            att_group(1, 0, g)
        qproj(1)
        qdup(1)
        normalize(1, 0)
        for g in range(NG):
            att_group(2, 0, g)
        normalize(2, 0)

        # ---- qc 1..7: steady state ------------------------------------
        for qc in range(1, NSB):
            for h in range(HPC):
                for g in range(NG):
                    att_group(h, qc, g)
                    # previous q-chunk's output projection during h0
                    if h == 0 and g in (2, 6, 10, 14):
                        op_pair(qc - 1, (g - 2) // 4)
                    # next q-chunk's projection during h1
                    if h == 1 and qc + 1 < NSB:
                        if g == 2:
                            qx_dma(qc + 1)
                        elif g == 8:
                            qproj(qc + 1)
                            qdup(qc + 1)
                normalize(h, qc)
        for i in range(4):
            op_pair(NSB - 1, i)


_NC_CACHE = {}


def _build():
    if "nc" not in _NC_CACHE:
        nc = bacc.Bacc(
            "TRN2", target_bir_lowering=False, debug=False, num_devices=NC_CORES
        )
        with tile.TileContext(nc) as tc:
            _emit(tc)
        nc.compile()
        _NC_CACHE["nc"] = nc
    return _NC_CACHE["nc"]


def _tile_xT(x):
    # x: [S, DM] fp32 -> x.T tiled as [KC, NSB, 128, 512] fp16 so each
    # (kc, sb) DMA slice is one contiguous 128 KiB block
    xT = np.ascontiguousarray(x.T).astype(np.float16)  # [DM, S]
    t = xT.reshape(KC, 128, NSB, 512).transpose(0, 2, 1, 3)
    return np.ascontiguousarray(t)


def make_in_maps(query, key, value, wq, bq, wk, bk, wv, bv, wo, bo):
    query = np.asarray(query)
    key = np.asarray(key)
    value = np.asarray(value)
    wq, bq, wk, bk, wv, bv, wo, bo = (
        np.asarray(a) for a in (wq, bq, wk, bk, wv, bv, wo, bo)
    )
    in_maps = []
    for c in range(NC_CORES):
        b = c // 4
        hs = (c % 4) * HPC * DK
        he = hs + HPC * DK
        woT = np.ascontiguousarray(wo[:, hs:he].T).astype(np.float16)  # [192, 768]
        wo2d = np.concatenate([woT[128:192], woT[128:192]], 0)  # [128, 768]
        in_maps.append(
            {
                "qTx": _tile_xT(query[b]),
                "kTx": _tile_xT(key[b]),
                "vTx": _tile_xT(value[b]),
                "wqT": np.ascontiguousarray(
                    (wq[hs:he, :] * QPRE).T
                ).astype(np.float16),
                "wkT": np.ascontiguousarray(wk[hs:he, :].T).astype(np.float16),
                "wvT": np.ascontiguousarray(wv[hs:he, :].T).astype(np.float16),
                "wo01": np.ascontiguousarray(woT[0:128]),
                "wo2d": np.ascontiguousarray(wo2d),
                "bq": (bq[hs:he] * QPRE).reshape(-1, 1).astype(np.float32),
                "bk": bk[hs:he].reshape(-1, 1).astype(np.float32),
                "bv": bv[hs:he].reshape(-1, 1).astype(np.float32),
            }
        )
    return in_maps


def combine_outputs(results, bo):
    parts = [results[c]["out_p"] for c in range(NC_CORES)]
    out0 = parts[0] + parts[1] + parts[2] + parts[3]
    out1 = parts[4] + parts[5] + parts[6] + parts[7]
    out = np.stack([out0, out1]) + np.asarray(bo)[None, None, :]
    return out.astype(np.float32)


def run_on_hw(in_maps, **kw):
    nc = _build()
    return run_bass_kernel_spmd(nc, in_maps, list(range(NC_CORES)), **kw)


def kernel(query, key, value, wq, bq, wk, bk, wv, bv, wo, bo):
    in_maps = make_in_maps(query, key, value, wq, bq, wk, bk, wv, bv, wo, bo)
    res = run_on_hw(in_maps)
    return combine_outputs(res.results, bo)


# revision 8
# speedup vs baseline: 1.2458x; 1.2458x over previous
"""Multi-head attention (B=2, S=4096, D=768, H=12) on 8 TRN2 NeuronCores.

Sharding: 24 (batch, head) pairs -> 3 heads per core. Cores 0-3 take batch 0,
cores 4-7 take batch 1. Each core computes q/k/v projections for its 3 heads,
flash-style attention (scores kept transposed [kv, q]), and a partial output
projection over its 192 contraction rows. The host sums the 4 partial outputs
per batch and adds the output bias.

Softmax exp is split across two engines: the Scalar engine (ACT LUT exp) and
a custom Vector-engine op EXP8C that evaluates exp(8v) as a factored minimax
cubic for e^v followed by three squarings. Scores are produced pre-scaled
(host folds g/64 into wq/bq, g = c3^(1/3) of the cubic) so both paths read the
same PSUM tile: ACT applies scale 8/g in its free affine; the DVE op's cubic
constants absorb g.

The k/v projections are pipelined into the first q-chunk's attention pass
(per-seq-block just-in-time), so the Scalar/Vector engines start exp work
almost immediately instead of idling through a serial projection prefix.
Softmax denominators ride a ones-column through the PV matmul, are broadcast
across partitions with a tiny K=1 matmul (no DRAM bounce), inverted with the
fast DVE reciprocal, and applied to the context before the output projection.
"""

import sys

sys.path.insert(0, "/opt/trn_rl_repo")

import numpy as np  # noqa: E402

from concourse import bacc, bass, mybir, tile  # noqa: E402
from concourse import dve_ops  # noqa: E402
from concourse.bass_utils import run_bass_kernel_spmd  # noqa: E402
from concourse.dve_spec import C0, C1, C2, Spec, Src0, lower  # noqa: E402
from concourse.dve_uop import DveOpSpec  # noqa: E402

S = 4096
DM = 768
DK = 64
HPC = 3  # heads per core
NC_CORES = 8
KC = DM // 128  # 6 contraction chunks for projections
NSB = S // 512  # 8 seq blocks (projection N / attention q chunks)
NKV = S // 128  # 32 kv chunks
NG = 16  # kv groups of 2 per (head, q-chunk)

# exp(8v) ~ ((vt-EA)(vt^2+EB*vt+EC))^8 with vt = GEXP*v, v = s_raw/64.
# Minimax cubic c(v) ~ e^v on [-0.5, 0.5] (rel err 3.2e-4), factored as
# c3*(v-rho)(v^2+b*v+cq); GEXP = c3^(1/3) distributed into the factors.
GEXP = 0.5480315395002889
EA = -0.9028239  # GEXP * rho
EB = 0.79605097  # GEXP * b
EC = 1.1073068  # GEXP^2 * cq
ACT_SCALE = 8.0 / GEXP  # Scalar-engine exp free-affine scale
QPRE = GEXP / 64.0  # host-side wq/bq prescale

F16 = mybir.dt.float16
F32 = mybir.dt.float32


def _exp8c_ref(in0, in1, s0, s1, imm2):
    x = np.asarray(in0, np.float32)
    fa = (x - np.float32(s0)).astype(np.float32)
    f5 = (x * (x + np.float32(s1)) + np.float32(imm2)).astype(np.float32)
    p = (fa * f5).astype(np.float32)
    for _ in range(3):
        p = (p * p).astype(np.float32)
    return p


def _register_exp8c():
    name = "EXP8C_ANT"
    if name in dve_ops._SUB_OPCODE_FOR_NAME:
        return next(op for op in dve_ops.OPS if op.name == name)
    fa = Src0 - C0
    f5 = (Src0 * (Src0 + C1)) + C2
    f6 = fa * f5
    s1 = f6 * f6
    s2 = s1 * s1
    body = s2 * s2
    spec = Spec(body=body, reference=_exp8c_ref)
    row = max(dve_ops._SUB_OPCODE_FOR_NAME.values()) + 1
    assert row < 0x20, "no free custom-DVE opcode row"
    shas = {}
    for ver in ("v3", "v4"):
        uops = lower(spec, ver=ver)
        shas[ver] = DveOpSpec(
            name=name, opcode=row, uops=uops, rd1_en=False
        ).sha(ver)
    op = dve_ops.DveOp(name, spec, subdim=False, uops_sha=shas)
    dve_ops.OPS.append(op)
    dve_ops.CUSTOM_DVE_SPECS[name] = spec
    dve_ops._SUB_OPCODE_FOR_NAME[name] = row
    return op


EXP8C = _register_exp8c()

# kv chunks whose exp runs on the Vector engine (11 of 32): c % 3 == 2
DVE_CHUNKS = frozenset(c for c in range(NKV) if c % 3 == 2)


def _emit(tc):
    nc = tc.nc
    qTx = nc.dram_tensor("qTx", [KC, NSB, 128, 512], F16, kind="ExternalInput").ap()
    kTx = nc.dram_tensor("kTx", [KC, NSB, 128, 512], F16, kind="ExternalInput").ap()
    vTx = nc.dram_tensor("vTx", [KC, NSB, 128, 512], F16, kind="ExternalInput").ap()
    wqT = nc.dram_tensor("wqT", [DM, HPC * DK], F16, kind="ExternalInput").ap()
    wkT = nc.dram_tensor("wkT", [DM, HPC * DK], F16, kind="ExternalInput").ap()
    wvT = nc.dram_tensor("wvT", [DM, HPC * DK], F16, kind="ExternalInput").ap()
    wo01 = nc.dram_tensor("wo01", [128, DM], F16, kind="ExternalInput").ap()
    wo2d_d = nc.dram_tensor("wo2d", [128, DM], F16, kind="ExternalInput").ap()
    bq = nc.dram_tensor("bq", [HPC * DK, 1], F32, kind="ExternalInput").ap()
    bk = nc.dram_tensor("bk", [HPC * DK, 1], F32, kind="ExternalInput").ap()
    bv = nc.dram_tensor("bv", [HPC * DK, 1], F32, kind="ExternalInput").ap()
    out_p = nc.dram_tensor("out_p", [S, DM], F32, kind="ExternalOutput").ap()

    with (
        tc.tile_pool(name="const", bufs=1) as const,
        tc.tile_pool(name="heads", bufs=1) as heads,
        tc.tile_pool(name="xts", bufs=12) as xts,
        tc.tile_pool(name="work", bufs=3) as work,
        tc.tile_pool(name="norm", bufs=2) as norm,
        tc.tile_pool(name="sp", bufs=4, space=bass.MemorySpace.PSUM) as sp,
        tc.tile_pool(name="cxp", bufs=2, space=bass.MemorySpace.PSUM) as cxp,
        tc.tile_pool(name="aux", bufs=2, space=bass.MemorySpace.PSUM) as aux,
    ):
        # ---- constants -------------------------------------------------
        w_q = const.tile([128, KC, HPC * DK], F16, tag="w_q")
        w_k = const.tile([128, KC, HPC * DK], F16, tag="w_k")
        w_v = const.tile([128, KC, HPC * DK], F16, tag="w_v")
        wo01_t = const.tile([128, DM], F16, tag="wo01")
        wo2d_t = const.tile([128, DM], F16, tag="wo2d")
        bq01 = const.tile([128, 1], F32, tag="bq01")
        bq2 = const.tile([DK, 1], F32, tag="bq2")
        bk01 = const.tile([128, 1], F32, tag="bk01")
        bk2 = const.tile([DK, 1], F32, tag="bk2")
        bvb = const.tile([128, HPC * DK], F32, tag="bvb")
        ones16 = const.tile([1, DK], F16, tag="ones16")
        bv_bcast = bass.AP(
            tensor=bv.tensor, offset=bv.offset, ap=[[0, 128]] + list(bv.ap)
        )

        nc.sync.dma_start(w_k[:], wkT.rearrange("(c p) m -> p c m", p=128))
        nc.sync.dma_start(bk01[:], bk[0:128, :])
        nc.sync.dma_start(bk2[:], bk[128:192, :])
        nc.sync.dma_start(w_q[:], wqT.rearrange("(c p) m -> p c m", p=128))
        nc.sync.dma_start(bq01[:], bq[0:128, :])
        nc.sync.dma_start(bq2[:], bq[128:192, :])
        nc.vector.memset(ones16[:], 1.0)

        # preload the exp activation table early
        warm = const.tile([1, 1], F32, tag="warm")
        nc.vector.memset(warm[:], 0.0)
        nc.scalar.activation(warm[:], warm[:], mybir.ActivationFunctionType.Exp)

        # ---- per-head persistent tensors ------------------------------
        # qT2/kT2: [128, S] fp16, rows 0:64 and 64:128 duplicated so
        # row-tiled matmul pairs can stream rhs from both partition halves.
        qT2 = [heads.tile([128, S], F16, tag=f"qT2_{h}", name=f"qT2_{h}") for h in range(HPC)]
        kT2 = [heads.tile([128, S], F16, tag=f"kT2_{h}", name=f"kT2_{h}") for h in range(HPC)]
        # v_aug: [128, NKV*65]; group c cols [65c, 65c+64) = v rows of
        # kv-chunk c, col 65c+64 = 1.0 (denominator column).
        v_aug = [heads.tile([128, NKV * 65], F16, tag=f"v_aug_{h}", name=f"v_aug_{h}") for h in range(HPC)]
        # normalized context, transposed: ctx01 rows 0:64 = head 0, rows
        # 64:128 = head 1; ctx2d rows 0:64 == rows 64:128 = head 2
        # (duplicated for the paired output projection).
        ctx01 = heads.tile([128, S], F16, tag="ctx01")
        ctx2d = heads.tile([128, S], F16, tag="ctx2d")

        for h in range(HPC):
            nc.vector.memset(v_aug[h][:], 1.0)

        nc.sync.dma_start(w_v[:], wvT.rearrange("(c p) m -> p c m", p=128))
        nc.sync.dma_start(bvb[:], bv_bcast)
        nc.sync.dma_start(wo01_t[:], wo01)
        nc.sync.dma_start(wo2d_t[:], wo2d_d)

        # ---- input tile DMA helpers -----------------------------------
        kx_tiles = {}
        vx_tiles = {}
        qx_tiles = {}

        def kx_dma(s):
            kx_tiles[s] = []
            for kc in range(KC):
                t = xts.tile([128, 512], F16, tag="kx", bufs=12, name=f"kx_{s}_{kc}")
                nc.sync.dma_start(t[:], kTx[kc, s])
                kx_tiles[s].append(t)

        def vx_dma(s):
            vx_tiles[s] = []
            for kc in range(KC):
                t = xts.tile([128, 512], F16, tag="vx", bufs=12, name=f"vx_{s}_{kc}")
                nc.sync.dma_start(t[:], vTx[kc, s])
                vx_tiles[s].append(t)

        def qx_dma(qc):
            qx_tiles[qc] = []
            for kc in range(KC):
                t = xts.tile([128, 512], F16, tag="qx", bufs=12, name=f"qx_{qc}_{kc}")
                nc.sync.dma_start(t[:], qTx[kc, qc])
                qx_tiles[qc].append(t)

        # ---- projection helpers ---------------------------------------
        def kproj(s):
            # heads 0/1: one aux pass, M=128
            sq = bass.ts(s, 512)
            t01 = aux.tile([128, 512], F32, tag="aux", name=f"k01_{s}")
            for kc in range(KC):
                nc.tensor.matmul(
                    t01[:], w_k[:, kc, 0:128], kx_tiles[s][kc][:],
                    start=(kc == 0), stop=(kc == KC - 1),
                )
            nc.vector.tensor_scalar_add(kT2[0][0:64, sq], t01[0:64, :], bk01[0:64, :])
            nc.vector.tensor_scalar_add(kT2[1][0:64, sq], t01[64:128, :], bk01[64:128, :])
            # head 2: second aux pass, M=64
            t2 = aux.tile([128, 512], F32, tag="aux", name=f"k2_{s}")
            for kc in range(KC):
                nc.tensor.matmul(
                    t2[0:64, :], w_k[:, kc, 128:192], kx_tiles[s][kc][:],
                    start=(kc == 0), stop=(kc == KC - 1),
                )
            nc.vector.tensor_scalar_add(kT2[2][0:64, sq], t2[0:64, :], bk2[:])
            for h in range(HPC):
                nc.sync.dma_start(kT2[h][64:128, sq], kT2[h][0:64, sq])

        def vproj(c):
            # kv chunk c: [128 seq, 192 dims] via aux; seq slice within sb
            s, ss = c // 4, c % 4
            vp = aux.tile([128, 512], F32, tag="aux", name=f"vp_{c}")
            for kc in range(KC):
                nc.tensor.matmul(
                    vp[:, 0:HPC * DK],
                    vx_tiles[s][kc][:, bass.ts(ss, 128)],
                    w_v[:, kc, :],
                    start=(kc == 0), stop=(kc == KC - 1),
                )
            g = c * 65
            for h in range(HPC):
                nc.vector.tensor_add(
                    v_aug[h][:, g : g + 64],
                    vp[:, bass.ts(h, 64)],
                    bvb[:, bass.ts(h, 64)],
                )

        def qproj(qc):
            sq = bass.ts(qc, 512)
            t01 = aux.tile([128, 512], F32, tag="aux", name=f"q01_{qc}")
            for kc in range(KC):
                nc.tensor.matmul(
                    t01[:], w_q[:, kc, 0:128], qx_tiles[qc][kc][:],
                    start=(kc == 0), stop=(kc == KC - 1),
                )
            nc.vector.tensor_scalar_add(qT2[0][0:64, sq], t01[0:64, :], bq01[0:64, :])
            nc.vector.tensor_scalar_add(qT2[1][0:64, sq], t01[64:128, :], bq01[64:128, :])
            t2 = aux.tile([128, 512], F32, tag="aux", name=f"q2_{qc}")
            for kc in range(KC):
                nc.tensor.matmul(
                    t2[0:64, :], w_q[:, kc, 128:192], qx_tiles[qc][kc][:],
                    start=(kc == 0), stop=(kc == KC - 1),
                )
            nc.vector.tensor_scalar_add(qT2[2][0:64, sq], t2[0:64, :], bq2[:])

        # ---- attention helpers ----------------------------------------
        ctx_state = {}

        def att_chunk(h, qc, c):
            # one kv chunk: score matmul (row-tiled by chunk parity), exp on
            # ACT or DVE, PV accumulate into the (h, qc) ctx tile.
            sq = bass.ts(qc, 512)
            if c == 0:
                ctx_state[(h, qc)] = cxp.tile([128, 512], F32, tag="cx", name=f"cx_{h}_{qc}")
            ctx = ctx_state[(h, qc)][0:65, :]
            sT = sp.tile([128, 512], F32, tag="sT", name=f"sT_{h}_{qc}_{c}")
            lo = 64 * (c % 2)
            nc.tensor.matmul(
                sT[:],
                kT2[h][lo : lo + 64, bass.ts(c, 128)],
                qT2[h][lo : lo + 64, sq],
            )
            pt = work.tile([128, 512], F16, tag="pt", bufs=8)
            if c in DVE_CHUNKS:
                nc.vector._custom_dve(
                    EXP8C, out=pt[:], in0=sT[:], s0=EA, s1=EB, imm2=EC
                )
            else:
                nc.scalar.activation(
                    pt[:], sT[:], mybir.ActivationFunctionType.Exp, scale=ACT_SCALE
                )
            gg = c * 65
            nc.tensor.matmul(
                ctx,
                v_aug[h][:, gg : gg + 65],
                pt[:],
                start=(c == 0),
                stop=(c == NKV - 1),
            )

        def att_group(h, qc, g):
            att_chunk(h, qc, 2 * g)
            att_chunk(h, qc, 2 * g + 1)

        def normalize(h, qc):
            # denominator row -> [1,512] f16 -> K=1 matmul broadcast to 64
            # partitions -> fast reciprocal -> scale ctx into SBUF fp16.
            sq = bass.ts(qc, 512)
            ctx = ctx_state.pop((h, qc))
            den16 = norm.tile([1, 512], F16, tag="den16")
            nc.vector.tensor_copy(den16[:], ctx[64:65, :])
            dbc = aux.tile([128, 512], F32, tag="aux", name=f"dbc_{h}_{qc}")
            nc.tensor.matmul(dbc[0:64, :], ones16[:], den16[:])
            rec = norm.tile([64, 512], F32, tag="rec")
            nc.vector.reciprocal_approx_fast(out=rec[:], in_=dbc[0:64, :])
            if h == 0:
                nc.vector.tensor_mul(ctx01[0:64, sq], ctx[0:64, :], rec[:])
            elif h == 1:
                nc.vector.tensor_mul(ctx01[64:128, sq], ctx[0:64, :], rec[:])
            else:
                nc.vector.tensor_mul(ctx2d[0:64, sq], ctx[0:64, :], rec[:])
                nc.vector.tensor_mul(ctx2d[64:128, sq], ctx[0:64, :], rec[:])

        def op_pair(qc, i):
            # chains 2i and 2i+1 of q-chunk qc's output projection; the two
            # head-2 matmuls (K=64) run as a concurrent row-tiled pair.
            chains = []
            for j in range(2):
                ci = 2 * i + j
                qs, half = ci // 2, ci % 2
                n0, nw = (0, 512) if half == 0 else (512, 256)
                qsl = bass.ds(qc * 512 + qs * 128, 128)
                op = aux.tile([128, 512], F32, tag="aux", name=f"op_{qc}_{ci}")
                nc.tensor.matmul(
                    op[:, 0:nw], ctx01[:, qsl], wo01_t[:, n0 : n0 + nw],
                    start=True, stop=False,
                )
                chains.append((op, qsl, n0, nw))
            for j in range(2):
                op, qsl, n0, nw = chains[j]
                lo = 64 * j
                nc.tensor.matmul(
                    op[:, 0:nw],
                    ctx2d[lo : lo + 64, qsl],
                    wo2d_t[lo : lo + 64, n0 : n0 + nw],
                    start=False, stop=True,
                )
            for j in range(2):
                op, qsl, n0, nw = chains[j]
                ob = work.tile([128, 512], F32, tag="ob", bufs=3)
                nc.vector.tensor_copy(ob[:, 0:nw], op[:, 0:nw])
                nc.sync.dma_start(out_p[qsl, n0 : n0 + nw], ob[:, 0:nw])

        def qdup(qc):
            sq = bass.ts(qc, 512)
            for h in range(HPC):
                nc.sync.dma_start(qT2[h][64:128, sq], qT2[h][0:64, sq])

        # ---- qc 0: JIT k/v projection + attention h0 -------------------
        qx_dma(0)
        kx_dma(0)
        vx_dma(0)
        qproj(0)
        qdup(0)
        for s in range(NSB):
            if s + 1 < NSB:
                kx_dma(s + 1)
                vx_dma(s + 1)
            kproj(s)
            for c in range(4 * s, 4 * s + 4):
                vproj(c)
            att_group(0, 0, 2 * s)
            att_group(0, 0, 2 * s + 1)
        normalize(0, 0)
        qx_dma(1)
        for g in range(NG):
            att_group(1, 0, g)
        qproj(1)
        qdup(1)
        normalize(1, 0)
        for g in range(NG):
            att_group(2, 0, g)
        normalize(2, 0)

        # ---- qc 1..7: steady state ------------------------------------
        for qc in range(1, NSB):
            for h in range(HPC):
                for g in range(NG):
                    att_group(h, qc, g)
                    # previous q-chunk's output projection during h0
                    if h == 0 and g in (2, 6, 10, 14):
                        op_pair(qc - 1, (g - 2) // 4)
                    # next q-chunk's projection during h1
                    if h == 1 and qc + 1 < NSB:
                        if g == 2:
                            qx_dma(qc + 1)
                        elif g == 8:
                            qproj(qc + 1)
                            qdup(qc + 1)
                normalize(h, qc)
        for i in range(4):
            op_pair(NSB - 1, i)


_NC_CACHE = {}


def _build():
    if "nc" not in _NC_CACHE:
        nc = bacc.Bacc(
            "TRN2", target_bir_lowering=False, debug=False, num_devices=NC_CORES
        )
        with tile.TileContext(nc) as tc:
            _emit(tc)
        nc.compile()
        _NC_CACHE["nc"] = nc
    return _NC_CACHE["nc"]


def _tile_xT(x):
    # x: [S, DM] fp32 -> x.T tiled as [KC, NSB, 128, 512] fp16 so each
    # (kc, sb) DMA slice is one contiguous 128 KiB block
    xT = np.ascontiguousarray(x.T).astype(np.float16)  # [DM, S]
    t = xT.reshape(KC, 128, NSB, 512).transpose(0, 2, 1, 3)
    return np.ascontiguousarray(t)


def make_in_maps(query, key, value, wq, bq, wk, bk, wv, bv, wo, bo):
    query = np.asarray(query)
    key = np.asarray(key)
    value = np.asarray(value)
    wq, bq, wk, bk, wv, bv, wo, bo = (
        np.asarray(a) for a in (wq, bq, wk, bk, wv, bv, wo, bo)
    )
    in_maps = []
    for c in range(NC_CORES):
        b = c // 4
        hs = (c % 4) * HPC * DK
        he = hs + HPC * DK
        woT = np.ascontiguousarray(wo[:, hs:he].T).astype(np.float16)  # [192, 768]
        wo2d = np.concatenate([woT[128:192], woT[128:192]], 0)  # [128, 768]
        in_maps.append(
            {
                "qTx": _tile_xT(query[b]),
                "kTx": _tile_xT(key[b]),
                "vTx": _tile_xT(value[b]),
                "wqT": np.ascontiguousarray(
                    (wq[hs:he, :] * QPRE).T
                ).astype(np.float16),
                "wkT": np.ascontiguousarray(wk[hs:he, :].T).astype(np.float16),
                "wvT": np.ascontiguousarray(wv[hs:he, :].T).astype(np.float16),
                "wo01": np.ascontiguousarray(woT[0:128]),
                "wo2d": np.ascontiguousarray(wo2d),
                "bq": (bq[hs:he] * QPRE).reshape(-1, 1).astype(np.float32),
                "bk": bk[hs:he].reshape(-1, 1).astype(np.float32),
                "bv": bv[hs:he].reshape(-1, 1).astype(np.float32),
            }
        )
    return in_maps


def combine_outputs(results, bo):
    parts = [results[c]["out_p"] for c in range(NC_CORES)]
    out0 = parts[0] + parts[1] + parts[2] + parts[3]
    out1 = parts[4] + parts[5] + parts[6] + parts[7]
    out = np.stack([out0, out1]) + np.asarray(bo)[None, None, :]
    return out.astype(np.float32)


def run_on_hw(in_maps, **kw):
    nc = _build()
    return run_bass_kernel_spmd(nc, in_maps, list(range(NC_CORES)), **kw)


def kernel(query, key, value, wq, bq, wk, bk, wv, bv, wo, bo):
    in_maps = make_in_maps(query, key, value, wq, bq, wk, bk, wv, bv, wo, bo)
    res = run_on_hw(in_maps)
    return combine_outputs(res.results, bo)
